# revision 12
# baseline (speedup 1.0000x reference)
"""ConvCapsules2d Trainium2 kernel (Bass/Tile), SPMD over 8 NeuronCores.

Full problem:
  poses (16,32,16,14,14) f32, W (32,32,16,3,3) f32
  V[n,b,c,d,f,g,k,l] = W[b,c,d,k,l] * sum_p poses[n,b,p,2f+k,2g+l]
  V: (16,32,32,16,6,6,3,3) f32  (~340 MB full) -> pure store-bandwidth bound.

Sharding: data-parallel over batch N: core i computes n in [2i, 2i+2).

The 2e-2-of-absmax tolerance lets the device emit a low-precision output and
the host restore f32:
  * compute runs in fp16. tensor_tensor fp16 hits the DVE 2x_1p perf mode
    (2 elem/lane/cycle) only when every operand's innermost AP step is +-1 on
    a 2-byte dtype, so the (k,l) axis is padded 9->10: weight rows are
    (d, kl10) and s rows are (fg, kl10), making both broadcast operands
    innermost-dense runs of 10 (20B, 4B-aligned).
  * stores go out via SWDGE (gpsimd) DMA with an fp16->int8 cast: W is
    pre-scaled by 127/absmax(V) on the host (absmax is exact: V factorizes as
    max_{b,kl} max|W| * max|s|), so the int8 quantization error is <=0.5 LSB
    = 0.4% of absmax, well under the 2e-2 gate.  HBM write traffic drops 4x
    vs the f32 baseline (11.8 MB/core incl. kl-pad, stripped on host).
  * DVE is then the bottleneck, so the work is spread: the first of the 8
    multiply groups runs on GPSIMD (tensor_mul on the Q7 cores, in 8 chunks
    interleaved with the store issues so SWDGE descriptor generation isn't
    starved), the first summation level of the P-reduction happens inside the
    poses load itself (SWDGE accum_op=add quarters), and the 9 unfold copies
    are fused into one overlapping-window strided copy.

Per-core layout: SBUF partition q = n*64 + b*2 + mm, output channel
c = 4*g + 2*mm + clo (g in 0..7).  Free dims carry (clo, d, fg, kl10), so the
DRAM side of each store merges (clo, u) into one contiguous 11520-element run
and the AP stays within the 3-dim DMA limit; each of the 8 stores is
(128 part x 11520 elem) with outer spray count 64.
"""
import numpy as np

import concourse.bacc as bacc
import concourse.mybir as mybir
from concourse.tile import TileContext
from concourse.ap import AP
from concourse import bass_utils

# ---- problem constants (hardcoded per contest contract) ----
NTOT, B, P, H = 16, 32, 16, 14
C, D, K, S = 32, 16, 3, 2
F = (H - K) // S + 1          # 6
FF, KK = F * F, K * K         # 36, 9
KL = 10                       # kl padded 9 -> 10 for the DVE 2x mode
NCORES = 8
N = NTOT // NCORES            # 2 batches per core
NPART = 128
M = C // 2                    # 16 c-pairs
MG, MM = 8, 2                 # c = 4*g + 2*mm + clo
UNIT = D * FF * KL            # 5760 padded elements per (partition, c)
HH = H * H
Q4 = P * HH // 4              # 784: poses quarter (4 p-maps)

STORE_INT8 = True             # False -> fp16 stores (no host dequant scale)


def _emit_body(nc, tc, cpool, wpool, opool, poses, Wt, V_ap):
    """One full kernel body; V_ap is the (N, B, C, UNIT) dram target AP."""
    fp16 = mybir.dt.float16

    # ---- poses load folds the first P-reduction level: quarters accumulate
    # into one (128, 784) tile via the SDMA CCE adder.
    psum4 = wpool.tile([NPART, Q4], fp16, tag="psum4")
    nc.sync.dma_start(out=psum4[:], in_=poses.ap()[:, 0:Q4])
    for j in range(1, 4):
        nc.gpsimd.dma_start(out=psum4[:], in_=poses.ap()[:, j * Q4:(j + 1) * Q4],
                            accum_op=mybir.AluOpType.add)
    W_sb = cpool.tile([NPART, M * D * KL], fp16, tag="wsb")
    nc.sync.dma_start(out=W_sb[:], in_=Wt.ap())

    # ---- finish the P-sum: 784 -> 392 -> 196
    tmp = wpool.tile([NPART, HH * 2], fp16, tag="tmp")
    acc = wpool.tile([NPART, HH], fp16, tag="acc")
    nc.vector.tensor_add(out=tmp[:], in0=psum4[:, :HH * 2], in1=psum4[:, HH * 2:])
    nc.vector.tensor_add(out=acc[:], in0=tmp[:, :HH], in1=tmp[:, HH:])

    # ---- unfold to s2 (f, g, kl10) in ONE strided copy: the source AP uses
    # overlapping windows (offset = (2f+k)*14 + (2g+l)); pad lane kl=9 is
    # never read downstream of the host strip, so it can stay garbage.
    s2 = wpool.tile([NPART, FF * KL], fp16, tag="s2")
    a = acc[:]
    src = AP(a.tensor, a.offset,
             [[HH, NPART], [2 * H, F], [2, F], [H, K], [1, K]])
    d_ = s2[:]
    dst = AP(d_.tensor, d_.offset,
             [[FF * KL, NPART], [F * KL, F], [KL, F], [K, K], [1, K]])
    nc.vector.tensor_copy(out=dst, in_=src)

    # ---- multiply + store; group 0 runs on GPSIMD in 8 chunks interleaved
    # with the store issues (all share the POOL instruction stream).
    vap = V_ap.rearrange("n b (g mm clo) u -> g n b mm clo u",
                         g=MG, mm=MM, clo=2)
    w_all = W_sb[:].rearrange("q (g clo d kl) -> q g clo d kl",
                              g=MG, clo=2, d=D)
    s_bc = s2[:].rearrange("q (fg kl) -> q fg kl", kl=KL)[:, None, None, :, :] \
                .broadcast_to((NPART, 2, D, FF, KL))
    gout = opool.tile([NPART, 2 * UNIT], fp16, tag="gout")
    gout_v = gout[:].rearrange("q (clo d fg kl) -> q clo d fg kl",
                               clo=2, d=D, fg=FF)
    s_d4 = s2[:].rearrange("q (fg kl) -> q fg kl", kl=KL)[:, None, :, :] \
                .broadcast_to((NPART, 4, FF, KL))

    def gchunk(ci):
        # chunk ci: (clo, d-quarter): 4 d's x (fg, kl) with <=4-dim APs for Q7
        clo, dq = divmod(ci, 4)
        dsl = slice(dq * 4, dq * 4 + 4)
        nc.gpsimd.tensor_mul(
            out=gout_v[:, clo, dsl],
            in0=w_all[:, 0, clo, dsl, None, :].broadcast_to((NPART, 4, FF, KL)),
            in1=s_d4)

    gchunk(0)
    for g in range(1, MG):
        out_t = opool.tile([NPART, 2 * UNIT], fp16, tag="out")
        out_v = out_t[:].rearrange("q (clo d fg kl) -> q clo d fg kl",
                                   clo=2, d=D, fg=FF)
        w_view = w_all[:, g, :, :, None, :].broadcast_to((NPART, 2, D, FF, KL))
        nc.vector.tensor_mul(out=out_v, in0=w_view, in1=s_bc)
        if STORE_INT8:
            nc.gpsimd.dma_start(out=vap[g], in_=out_t[:])  # fp16 -> int8 cast
        else:
            nc.sync.dma_start(out=vap[g], in_=out_t[:])
        gchunk(g)  # 8 chunks total: 1 pre-loop + 7 in-loop
    if STORE_INT8:
        nc.gpsimd.dma_start(out=vap[0], in_=gout[:])
    else:
        nc.sync.dma_start(out=vap[0], in_=gout[:])


def _build(nc):
    fp16 = mybir.dt.float16
    out_dt = mybir.dt.int8 if STORE_INT8 else fp16
    poses = nc.dram_tensor("poses", (NPART, P * HH), fp16, kind="ExternalInput")
    Wt = nc.dram_tensor("W", (NPART, M * D * KL), fp16, kind="ExternalInput")
    V = nc.dram_tensor("V", (N, B, C, UNIT), out_dt, kind="ExternalOutput")

    with TileContext(nc) as tc:
        with tc.tile_pool(name="const", bufs=1) as cpool, \
             tc.tile_pool(name="work", bufs=2) as wpool, \
             tc.tile_pool(name="out", bufs=3) as opool:
            _emit_body(nc, tc, cpool, wpool, opool, poses, Wt, V.ap())
    return nc


def _scale(W: np.ndarray, poses: np.ndarray) -> float:
    """Exact absmax of V (in f32 arithmetic): factorizes per (b, k, l)."""
    s = poses.sum(axis=2)                              # (NTOT, B, H, H)
    idx = (np.arange(F) * S)[:, None] + np.arange(K)[None, :]
    su = s[:, :, idx, :]                               # (NTOT,B,F,K,H)
    su = su[:, :, :, :, idx]                           # (NTOT,B,F,K,F,K)
    max_s = np.abs(su).transpose(1, 3, 5, 0, 2, 4).reshape(B, K, K, -1).max(axis=3)
    max_w = np.abs(W).transpose(0, 3, 4, 1, 2).reshape(B, K, K, -1).max(axis=3)
    return float((max_s * max_w).max())


def permute_W(W: np.ndarray) -> np.ndarray:
    """(B, C, D, K, K) f32 (pre-scaled) -> (128, M*D*KL) fp16, kl padded.

    Row q = n*64 + b*2 + mm holds W[b, 4g+2mm+clo, d, k, l] laid out as
    (g, clo, d, kl10).
    """
    Wp = W.reshape(B, MG, MM, 2, D, KK).transpose(0, 2, 1, 3, 4, 5)
    Wpad = np.zeros((B, MM, MG, 2, D, KL), dtype=np.float16)
    Wpad[..., :KK] = Wp.astype(np.float16)
    Wpad = Wpad.reshape(2 * B, M * D * KL)
    return np.ascontiguousarray(np.concatenate([Wpad, Wpad], axis=0))


def dup_poses(poses_shard: np.ndarray) -> np.ndarray:
    """(N, B, P, H, H) core shard -> (128, P*H*H) fp16: row n*64+b*2+mm."""
    flat = poses_shard.astype(np.float16).reshape(N, B, 1, P * HH)
    return np.ascontiguousarray(np.broadcast_to(flat, (N, B, 2, P * HH))
                                .reshape(NPART, P * HH))


_cached_nc = None


def _get_nc():
    global _cached_nc
    if _cached_nc is None:
        nc = bacc.Bacc("TRN2", target_bir_lowering=False)
        _build(nc)
        nc.compile()
        _cached_nc = nc
    return _cached_nc


def run_spmd(poses: np.ndarray, W: np.ndarray, **spmd_kwargs):
    """Shard, run on 8 cores, gather. Returns (V_full f32, BassKernelResults)."""
    poses = np.ascontiguousarray(np.asarray(poses, dtype=np.float32))
    W = np.ascontiguousarray(np.asarray(W, dtype=np.float32))
    assert poses.shape == (NTOT, B, P, H, H), poses.shape
    assert W.shape == (B, C, D, K, K), W.shape
    if STORE_INT8:
        A = _scale(W, poses)
        Wp = permute_W(W * (127.0 / A))
    else:
        A = None
        Wp = permute_W(W)
    nc = _get_nc()
    in_maps = [{"poses": dup_poses(poses[i * N:(i + 1) * N]), "W": Wp}
               for i in range(NCORES)]
    res = bass_utils.run_bass_kernel_spmd(nc, in_maps, core_ids=list(range(NCORES)),
                                          **spmd_kwargs)
    Vq = np.concatenate([r["V"] for r in res.results], axis=0)  # (16,B,C,UNIT)
    Vq = Vq.reshape(NTOT, B, C, D, FF, KL)[..., :KK]
    V = Vq.astype(np.float32)
    if STORE_INT8:
        V *= A / 127.0
    V = np.ascontiguousarray(V.reshape(NTOT, B, C, D, F, F, K, K))
    return V, res


def kernel(poses: np.ndarray, W: np.ndarray) -> np.ndarray:
    import time
    last_err = None
    for attempt in range(3):
        try:
            V, _ = run_spmd(poses, W)
            return V
        except Exception as e:  # transient NRT/axon device errors: poke + retry
            last_err = e
            time.sleep(2.0)
            try:
                import jax, jax.numpy as jnp
                jnp.sum(jnp.ones((8, 8))).block_until_ready()
            except Exception:
                pass
    raise last_err


# revision 16
# speedup vs baseline: 1.2710x; 1.2710x over previous
"""ConvCapsules2d Trainium2 kernel (Bass/Tile), SPMD over 8 NeuronCores.

Full problem:
  poses (16,32,16,14,14) f32, W (32,32,16,3,3) f32
  V[n,b,c,d,f,g,k,l] = W[b,c,d,k,l] * sum_p poses[n,b,p,2f+k,2g+l]
  V: (16,32,32,16,6,6,3,3) f32  (~340 MB full) -> pure store-bandwidth bound.

Sharding: data-parallel over batch N: core i computes n in [2i, 2i+2).

The 2e-2-of-absmax tolerance lets the device emit a low-precision output and
the host restore f32:
  * compute runs in fp16. tensor_tensor fp16 hits the DVE 2x_1p perf mode
    (2 elem/lane/cycle) only when every operand's innermost AP step is +-1 on
    a 2-byte dtype, so the (k,l) axis is padded 9->10: weight rows are
    (d, kl10) and s rows are (fg, kl10), making both broadcast operands
    innermost-dense runs of 10 (20B, 4B-aligned).
  * stores go out via SWDGE (gpsimd) DMA with an fp16->int8 cast: W is
    pre-scaled by 127/absmax(V) on the host (absmax is exact: V factorizes as
    max_{b,kl} max|W| * max|s|), so the int8 quantization error is <=0.5 LSB
    = 0.4% of absmax, well under the 2e-2 gate.  HBM write traffic drops 4x
    vs the f32 baseline (11.8 MB/core incl. kl-pad, stripped on host).
  * DVE is then the bottleneck, so the work is spread: the first of the 8
    multiply groups runs on GPSIMD (tensor_mul on the Q7 cores, in 8 chunks
    interleaved with the store issues so SWDGE descriptor generation isn't
    starved), the first summation level of the P-reduction happens inside the
    poses load itself (SWDGE accum_op=add quarters), and the 9 unfold copies
    are fused into one overlapping-window strided copy.

Per-core layout: SBUF partition q = n*64 + b*2 + mm, output channel
c = 4*g + 2*mm + clo (g in 0..7).  Free dims carry (clo, d, fg, kl10), so the
DRAM side of each store merges (clo, u) into one contiguous 11520-element run
and the AP stays within the 3-dim DMA limit; each of the 8 stores is
(128 part x 11520 elem) with outer spray count 64.
"""
import numpy as np

import concourse.bacc as bacc
import concourse.mybir as mybir
from concourse.tile import TileContext
from concourse.ap import AP
from concourse import bass_utils

# ---- problem constants (hardcoded per contest contract) ----
NTOT, B, P, H = 16, 32, 16, 14
C, D, K, S = 32, 16, 3, 2
F = (H - K) // S + 1          # 6
FF, KK = F * F, K * K         # 36, 9
KL = 10                       # kl padded 9 -> 10 for the DVE 2x mode
NCORES = 8
N = NTOT // NCORES            # 2 batches per core
NPART = 128
M = C // 2                    # 16 c-pairs
MG, MM = 8, 2                 # c = 4*g + 2*mm + clo
UNIT = D * FF * KL            # 5760 padded elements per (partition, c)
HH = H * H
Q4 = P * HH // 4              # 784: poses quarter (4 p-maps)

STORE_INT8 = True             # False -> fp16 stores (no host dequant scale)
GP_OFFLOAD = False             # group 0 multiply on GPSIMD instead of DVE
ACCUM_LOAD = True             # fold first P-sum level into the poses load


def _emit_body(nc, tc, cpool, wpool, opool, poses, Wt, V_ap):
    """One full kernel body; V_ap is the (N, B, C, UNIT) dram target AP."""
    fp16 = mybir.dt.float16

    # ---- poses load folds the first P-reduction level: quarters accumulate
    # into one (128, 784) tile via the SDMA CCE adder.
    acc = wpool.tile([NPART, HH], fp16, tag="acc")
    if ACCUM_LOAD:
        psum4 = wpool.tile([NPART, Q4], fp16, tag="psum4")
        nc.sync.dma_start(out=psum4[:], in_=poses.ap()[:, 0:Q4])
        for j in range(1, 4):
            nc.gpsimd.dma_start(out=psum4[:], in_=poses.ap()[:, j * Q4:(j + 1) * Q4],
                                accum_op=mybir.AluOpType.add)
        W_sb = cpool.tile([NPART, M * D * KL], fp16, tag="wsb")
        nc.sync.dma_start(out=W_sb[:], in_=Wt.ap())
        # ---- finish the P-sum: 784 -> 392 -> 196
        tmp = wpool.tile([NPART, HH * 2], fp16, tag="tmp")
        nc.vector.tensor_add(out=tmp[:], in0=psum4[:, :HH * 2], in1=psum4[:, HH * 2:])
        nc.vector.tensor_add(out=acc[:], in0=tmp[:, :HH], in1=tmp[:, HH:])
    else:
        poses_sb = cpool.tile([NPART, P * HH], fp16, tag="poses")
        nc.sync.dma_start(out=poses_sb[:], in_=poses.ap())
        W_sb = cpool.tile([NPART, M * D * KL], fp16, tag="wsb")
        nc.sync.dma_start(out=W_sb[:], in_=Wt.ap())
        tmp = wpool.tile([NPART, HH * 8], fp16, tag="tmp")
        nc.vector.tensor_add(out=tmp[:, :HH * 8],
                             in0=poses_sb[:, :HH * 8], in1=poses_sb[:, HH * 8:])
        nc.vector.tensor_add(out=tmp[:, :HH * 4],
                             in0=tmp[:, :HH * 4], in1=tmp[:, HH * 4:HH * 8])
        nc.vector.tensor_add(out=tmp[:, :HH * 2],
                             in0=tmp[:, :HH * 2], in1=tmp[:, HH * 2:HH * 4])
        nc.vector.tensor_add(out=acc[:], in0=tmp[:, :HH], in1=tmp[:, HH:HH * 2])

    # ---- unfold to s2 (f, g, kl10) in ONE strided copy: the source AP uses
    # overlapping windows (offset = (2f+k)*14 + (2g+l)); pad lane kl=9 is
    # never read downstream of the host strip, so it can stay garbage.
    s2 = wpool.tile([NPART, FF * KL], fp16, tag="s2")
    a = acc[:]
    src = AP(a.tensor, a.offset,
             [[HH, NPART], [2 * H, F], [2, F], [H, K], [1, K]])
    d_ = s2[:]
    dst = AP(d_.tensor, d_.offset,
             [[FF * KL, NPART], [F * KL, F], [KL, F], [K, K], [1, K]])
    nc.vector.tensor_copy(out=dst, in_=src)

    # ---- multiply + store; group 0 runs on GPSIMD in 8 chunks interleaved
    # with the store issues (all share the POOL instruction stream).
    vap = V_ap.rearrange("n b (g mm clo) u -> g n b mm clo u",
                         g=MG, mm=MM, clo=2)
    w_all = W_sb[:].rearrange("q (g clo d kl) -> q g clo d kl",
                              g=MG, clo=2, d=D)
    s_bc = s2[:].rearrange("q (fg kl) -> q fg kl", kl=KL)[:, None, None, :, :] \
                .broadcast_to((NPART, 2, D, FF, KL))
    if GP_OFFLOAD:
        gout = opool.tile([NPART, 2 * UNIT], fp16, tag="gout")
        gout_v = gout[:].rearrange("q (clo d fg kl) -> q clo d fg kl",
                                   clo=2, d=D, fg=FF)
        s_d4 = s2[:].rearrange("q (fg kl) -> q fg kl", kl=KL)[:, None, :, :] \
                    .broadcast_to((NPART, 4, FF, KL))

    def gchunk(ci):
        if not GP_OFFLOAD:
            return
        # chunk ci: (clo, d-quarter): 4 d's x (fg, kl) with <=4-dim APs for Q7
        clo, dq = divmod(ci, 4)
        dsl = slice(dq * 4, dq * 4 + 4)
        nc.gpsimd.tensor_mul(
            out=gout_v[:, clo, dsl],
            in0=w_all[:, 0, clo, dsl, None, :].broadcast_to((NPART, 4, FF, KL)),
            in1=s_d4)

    def store(g, tile):
        if STORE_INT8:
            nc.gpsimd.dma_start(out=vap[g], in_=tile[:])  # fp16 -> int8 cast
        else:
            nc.sync.dma_start(out=vap[g], in_=tile[:])

    gchunk(0)
    g0 = 1 if GP_OFFLOAD else 0
    for g in range(g0, MG):
        out_t = opool.tile([NPART, 2 * UNIT], fp16, tag="out")
        out_v = out_t[:].rearrange("q (clo d fg kl) -> q clo d fg kl",
                                   clo=2, d=D, fg=FF)
        w_view = w_all[:, g, :, :, None, :].broadcast_to((NPART, 2, D, FF, KL))
        nc.vector.tensor_mul(out=out_v, in0=w_view, in1=s_bc)
        store(g, out_t)
        gchunk(g)  # 8 chunks total: 1 pre-loop + 7 in-loop
    if GP_OFFLOAD:
        store(0, gout)


def _build(nc):
    fp16 = mybir.dt.float16
    out_dt = mybir.dt.int8 if STORE_INT8 else fp16
    poses = nc.dram_tensor("poses", (NPART, P * HH), fp16, kind="ExternalInput")
    Wt = nc.dram_tensor("W", (NPART, M * D * KL), fp16, kind="ExternalInput")
    V = nc.dram_tensor("V", (N, B, C, UNIT), out_dt, kind="ExternalOutput")

    with TileContext(nc) as tc:
        with tc.tile_pool(name="const", bufs=1) as cpool, \
             tc.tile_pool(name="work", bufs=2) as wpool, \
             tc.tile_pool(name="out", bufs=3) as opool:
            _emit_body(nc, tc, cpool, wpool, opool, poses, Wt, V.ap())
    return nc


def _scale(W: np.ndarray, poses: np.ndarray) -> float:
    """Exact absmax of V (in f32 arithmetic): factorizes per (b, k, l)."""
    s = poses.sum(axis=2)                              # (NTOT, B, H, H)
    idx = (np.arange(F) * S)[:, None] + np.arange(K)[None, :]
    su = s[:, :, idx, :]                               # (NTOT,B,F,K,H)
    su = su[:, :, :, :, idx]                           # (NTOT,B,F,K,F,K)
    max_s = np.abs(su).transpose(1, 3, 5, 0, 2, 4).reshape(B, K, K, -1).max(axis=3)
    max_w = np.abs(W).transpose(0, 3, 4, 1, 2).reshape(B, K, K, -1).max(axis=3)
    return float((max_s * max_w).max())


def permute_W(W: np.ndarray) -> np.ndarray:
    """(B, C, D, K, K) f32 (pre-scaled) -> (128, M*D*KL) fp16, kl padded.

    Row q = n*64 + b*2 + mm holds W[b, 4g+2mm+clo, d, k, l] laid out as
    (g, clo, d, kl10).
    """
    Wp = W.reshape(B, MG, MM, 2, D, KK).transpose(0, 2, 1, 3, 4, 5)
    Wpad = np.zeros((B, MM, MG, 2, D, KL), dtype=np.float16)
    Wpad[..., :KK] = Wp.astype(np.float16)
    Wpad = Wpad.reshape(2 * B, M * D * KL)
    return np.ascontiguousarray(np.concatenate([Wpad, Wpad], axis=0))


def dup_poses(poses_shard: np.ndarray) -> np.ndarray:
    """(N, B, P, H, H) core shard -> (128, P*H*H) fp16: row n*64+b*2+mm."""
    flat = poses_shard.astype(np.float16).reshape(N, B, 1, P * HH)
    return np.ascontiguousarray(np.broadcast_to(flat, (N, B, 2, P * HH))
                                .reshape(NPART, P * HH))


_cached_nc = None


def _get_nc():
    global _cached_nc
    if _cached_nc is None:
        nc = bacc.Bacc("TRN2", target_bir_lowering=False)
        _build(nc)
        nc.compile()
        _cached_nc = nc
    return _cached_nc


def run_spmd(poses: np.ndarray, W: np.ndarray, **spmd_kwargs):
    """Shard, run on 8 cores, gather. Returns (V_full f32, BassKernelResults)."""
    poses = np.ascontiguousarray(np.asarray(poses, dtype=np.float32))
    W = np.ascontiguousarray(np.asarray(W, dtype=np.float32))
    assert poses.shape == (NTOT, B, P, H, H), poses.shape
    assert W.shape == (B, C, D, K, K), W.shape
    if STORE_INT8:
        A = _scale(W, poses)
        Wp = permute_W(W * (127.0 / A))
    else:
        A = None
        Wp = permute_W(W)
    nc = _get_nc()
    in_maps = [{"poses": dup_poses(poses[i * N:(i + 1) * N]), "W": Wp}
               for i in range(NCORES)]
    res = bass_utils.run_bass_kernel_spmd(nc, in_maps, core_ids=list(range(NCORES)),
                                          **spmd_kwargs)
    Vq = np.concatenate([r["V"] for r in res.results], axis=0)  # (16,B,C,UNIT)
    Vq = Vq.reshape(NTOT, B, C, D, FF, KL)[..., :KK]
    V = Vq.astype(np.float32)
    if STORE_INT8:
        V *= A / 127.0
    V = np.ascontiguousarray(V.reshape(NTOT, B, C, D, F, F, K, K))
    return V, res


def kernel(poses: np.ndarray, W: np.ndarray) -> np.ndarray:
    import time
    last_err = None
    for attempt in range(3):
        try:
            V, _ = run_spmd(poses, W)
            return V
        except Exception as e:  # transient NRT/axon device errors: poke + retry
            last_err = e
            time.sleep(2.0)
            try:
                import jax, jax.numpy as jnp
                jnp.sum(jnp.ones((8, 8))).block_until_ready()
            except Exception:
                pass
    raise last_err


# revision 17
# speedup vs baseline: 1.4021x; 1.1032x over previous
"""ConvCapsules2d Trainium2 kernel (Bass/Tile), SPMD over 8 NeuronCores.

Full problem:
  poses (16,32,16,14,14) f32, W (32,32,16,3,3) f32
  V[n,b,c,d,f,g,k,l] = W[b,c,d,k,l] * sum_p poses[n,b,p,2f+k,2g+l]
  V: (16,32,32,16,6,6,3,3) f32  (~340 MB full) -> pure store-bandwidth bound.

Sharding: data-parallel over batch N: core i computes n in [2i, 2i+2).

The 2e-2-of-absmax tolerance lets the device emit a low-precision output and
the host restore f32:
  * compute runs in fp16. tensor_tensor fp16 hits the DVE 2x_1p perf mode
    (2 elem/lane/cycle) only when every operand's innermost AP step is +-1 on
    a 2-byte dtype, so the (k,l) axis is padded 9->10: weight rows are
    (d, kl10) and s rows are (fg, kl10), making both broadcast operands
    innermost-dense runs of 10 (20B, 4B-aligned).
  * stores go out via SWDGE (gpsimd) DMA with an fp16->int8 cast: W is
    pre-scaled by 127/absmax(V) on the host (absmax is exact: V factorizes as
    max_{b,kl} max|W| * max|s|), so the int8 quantization error is <=0.5 LSB
    = 0.4% of absmax, well under the 2e-2 gate.  HBM write traffic drops 4x
    vs the f32 baseline (11.8 MB/core incl. kl-pad, stripped on host).
  * DVE is then the bottleneck, so the work is spread: the first of the 8
    multiply groups runs on GPSIMD (tensor_mul on the Q7 cores, in 8 chunks
    interleaved with the store issues so SWDGE descriptor generation isn't
    starved), the first summation level of the P-reduction happens inside the
    poses load itself (SWDGE accum_op=add quarters), and the 9 unfold copies
    are fused into one overlapping-window strided copy.

Per-core layout: SBUF partition q = n*64 + b*2 + mm, output channel
c = 4*g + 2*mm + clo (g in 0..7).  Free dims carry (clo, d, fg, kl10), so the
DRAM side of each store merges (clo, u) into one contiguous 11520-element run
and the AP stays within the 3-dim DMA limit; each of the 8 stores is
(128 part x 11520 elem) with outer spray count 64.
"""
import numpy as np

import concourse.bacc as bacc
import concourse.mybir as mybir
from concourse.tile import TileContext
from concourse.ap import AP
from concourse import bass_utils

# ---- problem constants (hardcoded per contest contract) ----
NTOT, B, P, H = 16, 32, 16, 14
C, D, K, S = 32, 16, 3, 2
F = (H - K) // S + 1          # 6
FF, KK = F * F, K * K         # 36, 9
KL = 10                       # kl padded 9 -> 10 for the DVE 2x mode
NCORES = 8
N = NTOT // NCORES            # 2 batches per core
NPART = 128
M = C // 2                    # 16 c-pairs
MG, MM = 8, 2                 # c = 4*g + 2*mm + clo
UNIT = D * FF * KL            # 5760 padded elements per (partition, c)
HH = H * H
Q4 = P * HH // 4              # 784: poses quarter (4 p-maps)

STORE_INT8 = True             # False -> fp16 stores (no host dequant scale)
GP_OFFLOAD = False             # group 0 multiply on GPSIMD instead of DVE
ACCUM_LOAD = False             # fold first P-sum level into the poses load


def _emit_body(nc, tc, cpool, wpool, opool, poses, Wt, V_ap):
    """One full kernel body; V_ap is the (N, B, C, UNIT) dram target AP."""
    fp16 = mybir.dt.float16

    # ---- poses load folds the first P-reduction level: quarters accumulate
    # into one (128, 784) tile via the SDMA CCE adder.
    acc = wpool.tile([NPART, HH], fp16, tag="acc")
    if ACCUM_LOAD:
        psum4 = wpool.tile([NPART, Q4], fp16, tag="psum4")
        nc.sync.dma_start(out=psum4[:], in_=poses.ap()[:, 0:Q4])
        for j in range(1, 4):
            nc.gpsimd.dma_start(out=psum4[:], in_=poses.ap()[:, j * Q4:(j + 1) * Q4],
                                accum_op=mybir.AluOpType.add)
        W_sb = cpool.tile([NPART, M * D * KL], fp16, tag="wsb")
        nc.sync.dma_start(out=W_sb[:], in_=Wt.ap())
        # ---- finish the P-sum: 784 -> 392 -> 196
        tmp = wpool.tile([NPART, HH * 2], fp16, tag="tmp")
        nc.vector.tensor_add(out=tmp[:], in0=psum4[:, :HH * 2], in1=psum4[:, HH * 2:])
        nc.vector.tensor_add(out=acc[:], in0=tmp[:, :HH], in1=tmp[:, HH:])
    else:
        poses_sb = cpool.tile([NPART, P * HH], fp16, tag="poses")
        nc.sync.dma_start(out=poses_sb[:], in_=poses.ap())
        W_sb = cpool.tile([NPART, M * D * KL], fp16, tag="wsb")
        nc.sync.dma_start(out=W_sb[:], in_=Wt.ap())
        tmp = wpool.tile([NPART, HH * 8], fp16, tag="tmp")
        nc.vector.tensor_add(out=tmp[:, :HH * 8],
                             in0=poses_sb[:, :HH * 8], in1=poses_sb[:, HH * 8:])
        nc.vector.tensor_add(out=tmp[:, :HH * 4],
                             in0=tmp[:, :HH * 4], in1=tmp[:, HH * 4:HH * 8])
        nc.vector.tensor_add(out=tmp[:, :HH * 2],
                             in0=tmp[:, :HH * 2], in1=tmp[:, HH * 2:HH * 4])
        nc.vector.tensor_add(out=acc[:], in0=tmp[:, :HH], in1=tmp[:, HH:HH * 2])

    # ---- unfold to s2 (f, g, kl10) in ONE strided copy: the source AP uses
    # overlapping windows (offset = (2f+k)*14 + (2g+l)); pad lane kl=9 is
    # never read downstream of the host strip, so it can stay garbage.
    s2 = wpool.tile([NPART, FF * KL], fp16, tag="s2")
    a = acc[:]
    src = AP(a.tensor, a.offset,
             [[HH, NPART], [2 * H, F], [2, F], [H, K], [1, K]])
    d_ = s2[:]
    dst = AP(d_.tensor, d_.offset,
             [[FF * KL, NPART], [F * KL, F], [KL, F], [K, K], [1, K]])
    nc.vector.tensor_copy(out=dst, in_=src)

    # ---- multiply + store; group 0 runs on GPSIMD in 8 chunks interleaved
    # with the store issues (all share the POOL instruction stream).
    vap = V_ap.rearrange("n b (g mm clo) u -> g n b mm clo u",
                         g=MG, mm=MM, clo=2)
    w_all = W_sb[:].rearrange("q (g clo d kl) -> q g clo d kl",
                              g=MG, clo=2, d=D)
    s_bc = s2[:].rearrange("q (fg kl) -> q fg kl", kl=KL)[:, None, None, :, :] \
                .broadcast_to((NPART, 2, D, FF, KL))
    if GP_OFFLOAD:
        gout = opool.tile([NPART, 2 * UNIT], fp16, tag="gout")
        gout_v = gout[:].rearrange("q (clo d fg kl) -> q clo d fg kl",
                                   clo=2, d=D, fg=FF)
        s_d4 = s2[:].rearrange("q (fg kl) -> q fg kl", kl=KL)[:, None, :, :] \
                    .broadcast_to((NPART, 4, FF, KL))

    def gchunk(ci):
        if not GP_OFFLOAD:
            return
        # chunk ci: (clo, d-quarter): 4 d's x (fg, kl) with <=4-dim APs for Q7
        clo, dq = divmod(ci, 4)
        dsl = slice(dq * 4, dq * 4 + 4)
        nc.gpsimd.tensor_mul(
            out=gout_v[:, clo, dsl],
            in0=w_all[:, 0, clo, dsl, None, :].broadcast_to((NPART, 4, FF, KL)),
            in1=s_d4)

    def store(g, tile):
        if STORE_INT8:
            nc.gpsimd.dma_start(out=vap[g], in_=tile[:])  # fp16 -> int8 cast
        else:
            nc.sync.dma_start(out=vap[g], in_=tile[:])

    gchunk(0)
    g0 = 1 if GP_OFFLOAD else 0
    for g in range(g0, MG):
        out_t = opool.tile([NPART, 2 * UNIT], fp16, tag="out")
        out_v = out_t[:].rearrange("q (clo d fg kl) -> q clo d fg kl",
                                   clo=2, d=D, fg=FF)
        w_view = w_all[:, g, :, :, None, :].broadcast_to((NPART, 2, D, FF, KL))
        nc.vector.tensor_mul(out=out_v, in0=w_view, in1=s_bc)
        store(g, out_t)
        gchunk(g)  # 8 chunks total: 1 pre-loop + 7 in-loop
    if GP_OFFLOAD:
        store(0, gout)


def _build(nc):
    fp16 = mybir.dt.float16
    out_dt = mybir.dt.int8 if STORE_INT8 else fp16
    poses = nc.dram_tensor("poses", (NPART, P * HH), fp16, kind="ExternalInput")
    Wt = nc.dram_tensor("W", (NPART, M * D * KL), fp16, kind="ExternalInput")
    V = nc.dram_tensor("V", (N, B, C, UNIT), out_dt, kind="ExternalOutput")

    with TileContext(nc) as tc:
        with tc.tile_pool(name="const", bufs=1) as cpool, \
             tc.tile_pool(name="work", bufs=2) as wpool, \
             tc.tile_pool(name="out", bufs=3) as opool:
            _emit_body(nc, tc, cpool, wpool, opool, poses, Wt, V.ap())
    return nc


def _scale(W: np.ndarray, poses: np.ndarray) -> float:
    """Exact absmax of V (in f32 arithmetic): factorizes per (b, k, l)."""
    s = poses.sum(axis=2)                              # (NTOT, B, H, H)
    idx = (np.arange(F) * S)[:, None] + np.arange(K)[None, :]
    su = s[:, :, idx, :]                               # (NTOT,B,F,K,H)
    su = su[:, :, :, :, idx]                           # (NTOT,B,F,K,F,K)
    max_s = np.abs(su).transpose(1, 3, 5, 0, 2, 4).reshape(B, K, K, -1).max(axis=3)
    max_w = np.abs(W).transpose(0, 3, 4, 1, 2).reshape(B, K, K, -1).max(axis=3)
    return float((max_s * max_w).max())


def permute_W(W: np.ndarray) -> np.ndarray:
    """(B, C, D, K, K) f32 (pre-scaled) -> (128, M*D*KL) fp16, kl padded.

    Row q = n*64 + b*2 + mm holds W[b, 4g+2mm+clo, d, k, l] laid out as
    (g, clo, d, kl10).
    """
    Wp = W.reshape(B, MG, MM, 2, D, KK).transpose(0, 2, 1, 3, 4, 5)
    Wpad = np.zeros((B, MM, MG, 2, D, KL), dtype=np.float16)
    Wpad[..., :KK] = Wp.astype(np.float16)
    Wpad = Wpad.reshape(2 * B, M * D * KL)
    return np.ascontiguousarray(np.concatenate([Wpad, Wpad], axis=0))


def dup_poses(poses_shard: np.ndarray) -> np.ndarray:
    """(N, B, P, H, H) core shard -> (128, P*H*H) fp16: row n*64+b*2+mm."""
    flat = poses_shard.astype(np.float16).reshape(N, B, 1, P * HH)
    return np.ascontiguousarray(np.broadcast_to(flat, (N, B, 2, P * HH))
                                .reshape(NPART, P * HH))


_cached_nc = None


def _get_nc():
    global _cached_nc
    if _cached_nc is None:
        nc = bacc.Bacc("TRN2", target_bir_lowering=False)
        _build(nc)
        nc.compile()
        _cached_nc = nc
    return _cached_nc


def run_spmd(poses: np.ndarray, W: np.ndarray, **spmd_kwargs):
    """Shard, run on 8 cores, gather. Returns (V_full f32, BassKernelResults)."""
    poses = np.ascontiguousarray(np.asarray(poses, dtype=np.float32))
    W = np.ascontiguousarray(np.asarray(W, dtype=np.float32))
    assert poses.shape == (NTOT, B, P, H, H), poses.shape
    assert W.shape == (B, C, D, K, K), W.shape
    if STORE_INT8:
        A = _scale(W, poses)
        Wp = permute_W(W * (127.0 / A))
    else:
        A = None
        Wp = permute_W(W)
    nc = _get_nc()
    in_maps = [{"poses": dup_poses(poses[i * N:(i + 1) * N]), "W": Wp}
               for i in range(NCORES)]
    res = bass_utils.run_bass_kernel_spmd(nc, in_maps, core_ids=list(range(NCORES)),
                                          **spmd_kwargs)
    Vq = np.concatenate([r["V"] for r in res.results], axis=0)  # (16,B,C,UNIT)
    Vq = Vq.reshape(NTOT, B, C, D, FF, KL)[..., :KK]
    V = Vq.astype(np.float32)
    if STORE_INT8:
        V *= A / 127.0
    V = np.ascontiguousarray(V.reshape(NTOT, B, C, D, F, F, K, K))
    return V, res


def kernel(poses: np.ndarray, W: np.ndarray) -> np.ndarray:
    import time
    last_err = None
    for attempt in range(3):
        try:
            V, _ = run_spmd(poses, W)
            return V
        except Exception as e:  # transient NRT/axon device errors: poke + retry
            last_err = e
            time.sleep(2.0)
            try:
                import jax, jax.numpy as jnp
                jnp.sum(jnp.ones((8, 8))).block_until_ready()
            except Exception:
                pass
    raise last_err
